# revision 18
# baseline (speedup 1.0000x reference)
"""Local (windowed) attention with shared KV head — TRN2 Bass kernel, v4.

Problem: b=1, L=4096, d_model=1024, n_head=16, d_head=64, w=512.
  qp = (q@Wq)/8; k,v = kv@Wkv; per 512-chunk attention over {prev,self,next}
  chunks with zero-padded edges (softmax includes exp(0)=1 terms for pads);
  out = ctx @ Wo.

Sharding: sequence-parallel over the 8 chunks, one chunk per NeuronCore.
Each core recomputes the K/V projection for its 3-chunk halo (no collectives).

Design notes (evolved from the 381us baseline):
  - ScalarE exp is the roofline (~12.6M elements/core ~ 82+ us at 1
    elem/lane/cyc @1.2GHz + ~300cyc/instr overhead). The loop streams exp
    continuously: per (head-pair, y-block) one [128,1024] PSUM score tile
    (A|B halves via row-tiled concurrent K=64 matmuls at tile_position
    (0,0)/(64,0)) -> ONE wide ACTIVATE -> bf16 P tile -> two ctx matmuls
    (v65: 65th lhsT row of ones accumulates softmax denominator Z for free).
  - Scores double-buffered (2x2 PSUM banks) so ACT never waits on the PE.
  - bf16 datapath: halves DMA bytes/SBUF (PE rate unchanged, err ~5e-3).
  - reciprocal_approx_fast (base-partition-0 only!) on a packed [65,1024]
    tile replaces 4us-per-head iterative reciprocal.
  - Each dma_start costs ~600ns of *serialized submission* on its issuing
    engine's queue; submissions are spread across vector/gpsimd/scalar/sync
    so the ramp isn't gated by one queue.
  - q-projection for pair i+2 interleaves into pair i's groups; pair i-1's
    normalization matmuls defer into pair i's early groups (no PE
    head-of-line blocking of the ACT stream at pair boundaries).
  - out-projection is a dense tail; OUT is stored bf16 and widened on host.

PSUM banks in steady state: scores 2x[128,1024] (4) + cxA/cxB (2) + qp (1)
  + zbc (1) = 8.
"""

import numpy as np

B, L, DM, NH, DH, W = 1, 4096, 1024, 16, 64, 512
NCORES = 8
CH = L // NCORES        # 512 tokens per core
YW = 3 * W              # 1536 halo positions
P = 128
NF = DM // P            # 8 feature tiles
NY = YW // P            # 12 y blocks
NPAIR = NH // 2         # 8 head pairs

_CACHE = {}


def _build():
    import concourse.mybir as mybir
    import concourse.tile as tile
    from concourse import bacc
    from concourse.masks import make_identity
    from contextlib import ExitStack

    F32 = mybir.dt.float32
    BF16 = mybir.dt.bfloat16
    EXP = mybir.ActivationFunctionType.Exp

    nc = bacc.Bacc("TRN2", target_bir_lowering=False, debug=False)
    QT = nc.dram_tensor("QT", [DM, CH], BF16, kind="ExternalInput")
    KVT = nc.dram_tensor("KVT", [DM, YW], BF16, kind="ExternalInput")
    WQ = nc.dram_tensor("WQ", [DM, DM], BF16, kind="ExternalInput")     # pre-scaled by 1/8
    WVK = nc.dram_tensor("WVK", [DM, P], BF16, kind="ExternalInput")    # [Wv | Wk]
    WO = nc.dram_tensor("WO", [DM, DM], BF16, kind="ExternalInput")
    OUT = nc.dram_tensor("OUT", [CH, DM], BF16, kind="ExternalOutput")

    with tile.TileContext(nc) as tc, ExitStack() as ctx, \
         nc.allow_low_precision(reason="bf16 datapath; rel-err budget 2e-2"):
        perm = ctx.enter_context(tc.tile_pool(name="perm", bufs=1))

        identb = perm.tile([64, 64], BF16, tag="identb")
        make_identity(nc, identb[:])
        identF = perm.tile([P, P], BF16, tag="identF")
        make_identity(nc, identF[:])
        # ones row for the 1/Z broadcast matmuls (row 64 to stay lane-aligned
        # with the Z row of the ctx PSUM tiles)
        onesEb = perm.tile([65, 64], BF16, tag="onesEb")
        nc.vector.memset(onesEb[64:65, :], 1.0)

        # --- persistent SBUF tiles (bf16)
        wvk = [perm.tile([P, P], BF16, tag=f"wvk{f}", name=f"wvk{f}") for f in range(NF)]
        wq = [perm.tile([P, DM], BF16, tag=f"wq{f}", name=f"wq{f}") for f in range(NF)]
        wo = [perm.tile([P, DM], BF16, tag=f"wo{f}", name=f"wo{f}") for f in range(NF)]
        qt = [perm.tile([P, CH], BF16, tag=f"qt{f}", name=f"qt{f}") for f in range(NF)]
        k3T2 = perm.tile([P, YW], BF16, tag="k3T2")
        vTs = perm.tile([64, YW], BF16, tag="vTs")
        v65 = [perm.tile([P, 65], BF16, tag=f"v65_{t}", name=f"v65_{t}") for t in range(NY)]
        qpT = [perm.tile([P, CH], BF16, tag=f"qpT{m}", name=f"qpT{m}") for m in range(NF)]
        ctxn = [perm.tile([P, CH], BF16, tag=f"ctxn{i}", name=f"ctxn{i}") for i in range(NPAIR)]
        oacc = [perm.tile([P, W], BF16, tag=f"oacc{t}", name=f"oacc{t}") for t in range(8)]

        # SBUF placement pad: restores the P-pool address at which ACT exp
        # measured 1113ns instead of 1335ns (layout-sensitive overhead).
        sbpad = perm.tile([P, 6656], BF16, tag="sbpad")

        qpp = ctx.enter_context(tc.tile_pool(name="qpps", bufs=1, space="PSUM"))

        # ---------------- ramp: kv projection, v transposes, q projection
        with tc.tile_pool(name="kvt", bufs=1) as kvtp, \
             tc.tile_pool(name="ramp", bufs=2, space="PSUM") as rampp:
            kvt = [[kvtp.tile([P, W], BF16, tag=f"kvt{n}_{f}", name=f"kvt{n}_{f}") for f in range(NF)]
                   for n in range(3)]
            # DMA submissions cost ~600ns serialized on the issuing engine;
            # only sync (SP) and scalar are HWDGE (gpsimd is slow SWDGE).
            # Stripe each ramp-critical tensor across both queues.
            # All input DMAs on the sync queue in strict priority order:
            # submissions are credit-paced by the transfers themselves, and a
            # submission sitting in the scalar queue would block the exp
            # stream, so scalar must never carry DMAs.
            H = DM // 2
            for f in range(NF):
                nc.sync.dma_start(wvk[f][:], WVK.ap()[P * f:P * (f + 1), :])
            for f in range(NF):
                nc.sync.dma_start(kvt[0][f][:], KVT.ap()[P * f:P * (f + 1), 0:W])
            for f in range(NF):
                nc.sync.dma_start(qt[f][:], QT.ap()[P * f:P * (f + 1), :])
            for f in range(NF):
                nc.sync.dma_start(wq[f][:, 0:H], WQ.ap()[P * f:P * (f + 1), 0:H])
            for f in range(NF):
                nc.sync.dma_start(kvt[1][f][:], KVT.ap()[P * f:P * (f + 1), W:2 * W])
            for f in range(NF):
                nc.sync.dma_start(kvt[2][f][:],
                                  KVT.ap()[P * f:P * (f + 1), 2 * W:3 * W])

            def do_chunk(n):
                ps = rampp.tile([P, W], F32, tag="kvp", name="kvp")
                for f in range(NF):
                    nc.tensor.matmul(ps[:], wvk[f][:], kvt[n][f][:],
                                     start=(f == 0), stop=(f == NF - 1))
                ns = slice(W * n, W * (n + 1))
                nc.vector.tensor_copy(vTs[:, ns], ps[0:64, :])
                nc.vector.tensor_copy(k3T2[64:128, ns], ps[64:128, :])
                # duplicate kT into the low partition half (partition remap DMA)
                nc.sync.dma_start(k3T2[0:64, ns], k3T2[64:128, ns])
                for t in range(4 * n, 4 * n + 4):
                    tp = rampp.tile([P, 64], BF16, tag="tp", name="tp")
                    nc.tensor.transpose(tp[:], vTs[:, P * t:P * (t + 1)], identb[:])
                    nc.vector.tensor_copy(v65[t][:, 0:64], tp[:])
                    nc.vector.memset(v65[t][:, 64:65], 1.0)

            do_chunk(0)
            # q projection for pairs 0 and 1 (rest interleaved into the loop)
            for m in range(2):
                ps = qpp.tile([P, CH], F32, tag="qp", name="qp")
                for f in range(NF):
                    nc.tensor.matmul(ps[:], wq[f][:, P * m:P * (m + 1)], qt[f][:],
                                     start=(f == 0), stop=(f == NF - 1))
                nc.vector.tensor_copy(qpT[m][:], ps[:])
            do_chunk(1)
            do_chunk(2)
            # Deferred bulk DMAs: a [1,1] dummy copy from qpT[0] creates a WAW
            # dep so these transfers stay out of the DMA queues until the
            # ramp-critical set has landed.
            for f in range(NF):
                nc.vector.tensor_copy(wo[f][0:1, 0:1], qpT[0][0:1, 0:1])
                nc.sync.dma_start(wo[f][:], WO.ap()[P * f:P * (f + 1), :])
            for f in range(NF):
                nc.vector.tensor_copy(wq[f][0:1, H:H + 1], qpT[0][0:1, 0:1])
                nc.sync.dma_start(wq[f][:, H:DM],
                                  WQ.ap()[P * f:P * (f + 1), H:DM])

        # ---------------- attention main loop
        with tc.tile_pool(name="scps", bufs=2, space="PSUM") as scp, \
             tc.tile_pool(name="cxps", bufs=1, space="PSUM") as cxp, \
             tc.tile_pool(name="zbps", bufs=1, space="PSUM") as zbp, \
             tc.tile_pool(name="pt", bufs=3) as ptp, \
             tc.tile_pool(name="nrm", bufs=2) as nrm:

            pending_norm = [None]   # deferred zbc+mul emission for pair i-1

            def out_slices(t):
                x, o = t // 2, t % 2
                return slice(P * x, P * (x + 1)), slice(W * o, W * (o + 1))

            def emit_out_partial(t, jhi):
                # partial out-projection over pairs 0..jhi -> SBUF accumulator
                # (only ctxn[j] whose normalization is already emitted!)
                xs, os_ = out_slices(t)
                ps = qpp.tile([P, CH], F32, tag="qp", name=f"opp{t}")
                for j in range(jhi + 1):
                    nc.tensor.matmul(ps[:], ctxn[j][:, xs], wo[j][:, os_],
                                     start=(j == 0), stop=(j == jhi))
                nc.vector.tensor_copy(oacc[t][:], ps[:])

            def emit_out_mid(t, jlo, jhi):
                # pairs jlo..jhi + previous partial, accumulated on the PE via
                # an identity matmul (ps += I.T @ oacc) -- no DVE adds
                xs, os_ = out_slices(t)
                ps = qpp.tile([P, CH], F32, tag="qp", name=f"opj6{t}")
                nc.tensor.matmul(ps[:], identF[:], oacc[t][:],
                                 start=True, stop=False)
                for j in range(jlo, jhi + 1):
                    nc.tensor.matmul(ps[:], ctxn[j][:, xs], wo[j][:, os_],
                                     start=False, stop=(j == jhi))
                nc.vector.tensor_copy(oacc[t][:], ps[:])

            # (pair, group) -> (tile, jlo, jhi); jhi <= pair-1 always, and in
            # pairs 3-5 the qp psum bank is shared with qproj (alloc at g3,
            # freed at g11), so partials sit at g>=12 boundary-free slots g9/g11
            # is unsafe -- instead they go before qproj's alloc at g==1.
            OUT_SCHED = {
                (3, 1): (0, 0, 1), (3, 11): (1, 0, 2),
                (4, 1): (2, 0, 2), (4, 11): (3, 0, 3),
                (5, 1): (4, 0, 3), (5, 11): (5, 0, 4),
                (6, 3): (6, 0, 5), (6, 5): (7, 0, 5),
                (6, 7): (0, 2, 5), (6, 9): (1, 3, 5),
                (7, 3): (2, 3, 6), (7, 4): (3, 4, 6),
                (7, 5): (4, 4, 6), (7, 6): (5, 5, 6),
                (7, 7): (0, 6, 6), (7, 8): (1, 6, 6),
                (7, 9): (6, 6, 6), (7, 10): (7, 6, 6),
            }


            for i in range(NPAIR):
                cxA = cxp.tile([65, W], F32, tag="cxA")
                cxB = cxp.tile([65, W], F32, tag="cxB")
                pg = [None] * NY
                qp_ps = [None]

                def emit_ctx(g, cxA=cxA, cxB=cxB, pg=pg):
                    st, sp = (g == 0), (g == NY - 1)
                    nc.tensor.matmul(cxA[:, :], v65[g][:], pg[g][:, 0:W],
                                     start=st, stop=sp)
                    nc.tensor.matmul(cxB[:, :], v65[g][:], pg[g][:, W:2 * W],
                                     start=st, stop=sp)

                for g in range(NY):
                    ys = slice(P * g, P * (g + 1))
                    scS = scp.tile([P, 2 * W], F32, tag="sc")
                    nc.tensor.matmul(scS[:, 0:W], k3T2[0:64, ys],
                                     qpT[i][0:64, :], start=True, stop=True,
                                     tile_position=(0, 0))
                    nc.tensor.matmul(scS[:, W:2 * W], k3T2[64:128, ys],
                                     qpT[i][64:128, :], start=True, stop=True,
                                     tile_position=(64, 0))
                    pt_ = ptp.tile([P, 2 * W], BF16, tag="pt")
                    nc.scalar.activation(pt_[:], scS[:], EXP)
                    pg[g] = pt_

                    if g >= 1:
                        emit_ctx(g - 1)
                    if g == 2 and pending_norm[0] is not None:
                        pending_norm[0]()
                        pending_norm[0] = None
                    # q projection for pair i+2, one f-tile per group
                    m = i + 2
                    if m < NPAIR and 3 <= g <= 10:
                        f = g - 3
                        if f == 0:
                            qp_ps[0] = qpp.tile([P, CH], F32, tag="qp", name="qp2")
                        nc.tensor.matmul(qp_ps[0][:], wq[f][:, P * m:P * (m + 1)],
                                         qt[f][:], start=(f == 0), stop=(f == NF - 1))
                    if m < NPAIR and g == 11:
                        nc.vector.tensor_copy(qpT[m][:], qp_ps[0][:])
                    # out-projection partials ride the PE slack of pairs 3-7
                    # (at pair i, ctxn[0..i-1] are available after g==2)
                    ent = OUT_SCHED.get((i, g))
                    if ent is not None:
                        t, jlo, jhi = ent
                        if jlo == 0:
                            emit_out_partial(t, jhi)
                        else:
                            emit_out_mid(t, jlo, jhi)
                emit_ctx(NY - 1)

                # normalization prologue: evacuate Z + ctx from PSUM ASAP
                Zp = nrm.tile([65, 2 * W], F32, tag="Zp")
                zinv = nrm.tile([65, 2 * W], F32, tag="zinv")
                zinvb = nrm.tile([65, 2 * W], BF16, tag="zinvb")
                nc.vector.tensor_copy(Zp[64:65, 0:W], cxA[64:65, :])
                nc.vector.tensor_copy(Zp[64:65, W:2 * W], cxB[64:65, :])
                cxsA = nrm.tile([64, W], BF16, tag="cxsA")
                cxsB = nrm.tile([64, W], BF16, tag="cxsB")
                nc.vector.tensor_copy(cxsA[:], cxA[0:64, :])
                nc.vector.tensor_copy(cxsB[:], cxB[0:64, :])
                # custom DVE op requires base partition 0: run over all 65 rows
                # (rows 0:63 are don't-care lanes; row 64 holds Z_A|Z_B)
                nc.vector.reciprocal_approx_fast(zinv[:], Zp[:])
                nc.vector.tensor_copy(zinvb[64:65, :], zinv[64:65, :])

                def norm_tail(i=i, zinvb=zinvb, cxsA=cxsA, cxsB=cxsB):
                    zbA = zbp.tile([64, W], F32, tag="zb", name="zbA")
                    nc.tensor.matmul(zbA[:], onesEb[64:65, :], zinvb[64:65, 0:W],
                                     start=True, stop=True, tile_position=(64, 0))
                    nc.vector.tensor_mul(ctxn[i][0:64, :], cxsA[:], zbA[:])
                    zbB = zbp.tile([64, W], F32, tag="zb", name="zbB")
                    nc.tensor.matmul(zbB[:], onesEb[64:65, :], zinvb[64:65, W:2 * W],
                                     start=True, stop=True, tile_position=(64, 0))
                    cbt = nrm.tile([64, W], BF16, tag="cbt", name="cbt")
                    nc.vector.tensor_mul(cbt[:], cxsB[:], zbB[:])
                    nc.sync.dma_start(ctxn[i][64:128, :], cbt[:])

                pending_norm[0] = norm_tail

            pending_norm[0]()   # pair 7

        # ---------------- output projection tail: pair-7 contribution + add
        with tc.tile_pool(name="opps", bufs=4, space="PSUM") as opp, \
             tc.tile_pool(name="osb", bufs=4) as osb:
            for t in range(8):
                x, o = t // 2, t % 2
                xs = slice(P * x, P * (x + 1))
                os_ = slice(W * o, W * (o + 1))
                ps = opp.tile([P, W], F32, tag="op")
                nc.tensor.matmul(ps[:], identF[:], oacc[t][:],
                                 start=True, stop=False)
                nc.tensor.matmul(ps[:], ctxn[NPAIR - 1][:, xs],
                                 wo[NPAIR - 1][:, os_], start=False, stop=True)
                ot = osb.tile([P, W], BF16, tag="os")
                nc.scalar.copy(ot[:], ps[:])
                nc.sync.dma_start(OUT.ap()[xs, os_], ot[:])

    nc.compile()
    return nc


def _get_nc():
    if "nc" not in _CACHE:
        _CACHE["nc"] = _build()
    return _CACHE["nc"]


def kernel(q, kv, Wq, Wkv, Wo, w=None, _trace=False):
    import ml_dtypes
    from concourse import bass_utils

    BF = ml_dtypes.bfloat16

    q = np.asarray(q, np.float32).reshape(L, DM)
    kv = np.asarray(kv, np.float32).reshape(L, DM)
    Wq = np.asarray(Wq, np.float32)
    Wkv = np.asarray(Wkv, np.float32)
    Wo = np.asarray(Wo, np.float32)

    qT = np.ascontiguousarray(q.T.astype(BF))                 # [DM, L]
    kvT = np.ascontiguousarray(kv.T.astype(BF))               # [DM, L]
    WQs = np.ascontiguousarray((Wq / np.sqrt(DH)).astype(BF))  # fold 1/sqrt(d_head)
    WVK = np.ascontiguousarray(
        np.concatenate([Wkv[:, DH:], Wkv[:, :DH]], axis=1).astype(BF))  # [Wv | Wk]
    WOb = np.ascontiguousarray(Wo.astype(BF))

    in_maps = []
    for c in range(NCORES):
        kvt_c = np.zeros((DM, YW), BF)
        lo = (c - 1) * CH
        hi = (c + 2) * CH
        src_lo, src_hi = max(lo, 0), min(hi, L)
        dst_lo = src_lo - lo
        kvt_c[:, dst_lo:dst_lo + (src_hi - src_lo)] = kvT[:, src_lo:src_hi]
        in_maps.append({
            "QT": np.ascontiguousarray(qT[:, c * CH:(c + 1) * CH]),
            "KVT": kvt_c,
            "WQ": WQs,
            "WVK": WVK,
            "WO": WOb,
        })

    nc = _get_nc()
    res = bass_utils.run_bass_kernel_spmd(
        nc, in_maps, core_ids=list(range(NCORES)), trace=_trace)
    if _trace:
        _CACHE["last_result"] = res

    out = np.concatenate([np.asarray(r["OUT"]).astype(np.float32)
                          for r in res.results], axis=0)
    return out.reshape(B, L, DM)
